# revision 41
# baseline (speedup 1.0000x reference)
"""Trainium2 Bass kernel for CompositionModel (gnn_message_passing).

Model: per-cell MLP over [log1p(X) ++ Z[cell_to_batch]] followed by a
segment-mean over batch labels.

Strategy (v6 — one-hot segment table + variable slot capacities):
  * Each core owns 64 segments, sorted by cell count into 64 SLOTS with
    compiled capacities matched to the order statistics of the multinomial
    count distribution (caps rounded to 32, max 1024).  Overflow cells are
    evaluated exactly on the host.
  * Cell stream per sub-block interleaves [xl fp8 | one-hot fp8] columns:
    the one-hot column has ones at rows (2q, 2q+1) selecting slot q's
    zb hi/lo rows from a STATIC per-core weight table, so the L1 DoubleRow
    matmul adds the per-segment bias zb = Z @ W1z + b1 with no per-pair
    weight rewriting (no GPSIMD copies, no tile rotation).  W1 quantization
    error is mean-corrected through zb.
  * ACT: relu1 per sub-block (scale=1/64) -> fp8 h1.
  * L2: per (slot, half) two DoubleRow matmuls -> one fp32 PSUM tile;
    quantization mean-corrected per segment via b2c.
  * relu2 + segment sum: ONE op per (slot, half) with accum_out.  DVE path
    uses max(x+b,0) = max(x,-b)+b (single-source tensor_scalar, accumulator
    op add); host adds cap*b back.  Every ROT-th op runs on ACT (Relu +
    bias + accum) for engine balance, flushed one slot late.
  * The third (linear) layer commutes with the segment sum and is applied
    on the host; pad-cell contributions are subtracted analytically.
"""

import numpy as np
import ml_dtypes

import concourse.bacc as bacc
import concourse.mybir as mybir
import concourse.tile as tile
from concourse.bass_utils import run_bass_kernel_spmd

BF16 = ml_dtypes.bfloat16
FP8 = ml_dtypes.float8_e4m3fn

N_CORES = 8
DX = 128
DZ = 32
H = 256
B = 512
N_CELLS = 500_000
NSLOT = 64         # segments (slots) per core
SB = 4             # sub-blocks per superblock (one DMA)
NBLK = 2 * NSLOT   # sub-blocks per core
WSCALE = 64.0      # fp8 pre-scale on W1/W2/zb/b2, divided out at the end
ROT = 12           # every ROT-th relu2 op runs on ACT instead of DVE

_compiled = {}
_last_in_maps = None


def _slot_caps():
    """Compiled per-slot capacities: Blom-approximated expected order
    statistics (descending) of a core's 64 segment counts, rounded up to
    32, clamped to [64, 1024]."""
    from scipy.special import ndtri
    p = 1.0 / B
    mu = N_CELLS * p
    sig = np.sqrt(N_CELLS * p * (1 - p))
    q = np.arange(NSLOT, dtype=np.float64)          # 0 = largest
    r = NSLOT - q                                    # rank from smallest
    z = ndtri((r - 0.375) / (NSLOT + 0.25))
    caps = mu + sig * z
    caps = (np.round(caps / 32.0) * 32).astype(np.int64)
    return np.clip(caps, 64, 1024)


CAPS = _slot_caps()                    # [NSLOT] descending
C2 = CAPS // 2                         # per-sub-block cell count
SUB_W = 2 * C2                         # stream cols per sub-block (xl+onehot)
# sub-block i belongs to slot i//2; superblock j covers sub-blocks 4j..4j+3
SUB_OFF = np.concatenate([[0], np.cumsum(np.repeat(SUB_W, 2))])  # [NBLK+1]
TOTCOL = int(SUB_OFF[-1])
NSUPER = NBLK // SB
SUPER_OFF = SUB_OFF[::SB]              # [NSUPER+1]


def _build_program(nblk):
    f32 = mybir.dt.float32
    bf16 = mybir.dt.bfloat16
    fp8 = mybir.dt.float8e4
    Alu = mybir.AluOpType
    Act = mybir.ActivationFunctionType
    DR = mybir.MatmulPerfMode.DoubleRow
    assert nblk == NBLK
    npair = NSLOT

    nc = bacc.Bacc("TRN2", target_bir_lowering=False, debug=False,
                   num_devices=N_CORES)

    xt_d = nc.dram_tensor("xt", [DX, TOTCOL], fp8, kind="ExternalInput")
    # per half: [128, 256] = [64*W1x_hi | zb segment table (rows 2q,2q+1)]
    w1_d = nc.dram_tensor("w1", [2, 128, 256], fp8, kind="ExternalInput")
    w2hi_d = nc.dram_tensor("w2hi", [128, 512], fp8, kind="ExternalInput")
    mb_d = nc.dram_tensor("mb", [128, 2 * npair], f32, kind="ExternalInput")
    b2c_d = nc.dram_tensor("b2c", [128, 2 * npair], f32, kind="ExternalInput")
    out_d = nc.dram_tensor("out", [128, 2 * npair], f32, kind="ExternalOutput")

    with tile.TileContext(nc) as tc:
        with tc.tile_pool(name="consts", bufs=1) as cpool, \
             tc.tile_pool(name="h1", bufs=4) as h1pool, \
             tc.tile_pool(name="hsc", bufs=2) as hscpool, \
             tc.tile_pool(name="ps1", bufs=2, space="PSUM") as psum1, \
             tc.tile_pool(name="ps2", bufs=2, space="PSUM") as psum2:

            # startup DMAs: critical chain (xt0, w1) on the Sync queue;
            # the rest triggered from the idle GpSimd queue in parallel
            xts = [cpool.tile([DX, 4096], fp8, tag=f"xt{r}", name=f"xt{r}")
                   for r in range(3)]

            def load_super(j):
                if j >= NSUPER:
                    return
                o0, o1 = int(SUPER_OFF[j]), int(SUPER_OFF[j + 1])
                nc.sync.dma_start(xts[j % 3][:, 0:o1 - o0], xt_d[:, o0:o1])

            # w1 first (gates the first LDWEIGHTS), then superblock 0 split
            # across two triggers so its transfer spreads over two queues
            w1t = []
            for h in range(2):
                w = cpool.tile([128, 256], fp8, tag=f"w1_{h}",
                               name=f"w1_{h}")
                nc.sync.dma_start(w[:], w1_d[h])
                w1t.append(w)
            s0, s1 = int(SUPER_OFF[0]), int(SUPER_OFF[1])
            smid = s0 + ((s1 - s0) // 2 // 16) * 16
            nc.sync.dma_start(xts[0][:, 0:smid - s0], xt_d[:, s0:smid])
            nc.sync.dma_start(xts[0][:, smid - s0:s1 - s0], xt_d[:, smid:s1])
            w2 = cpool.tile([128, 512], fp8, tag="w2")
            nc.gpsimd.dma_start(w2[:], w2hi_d[:])
            w2hit = [w2[:, h * 256:(h + 1) * 256]
                     .rearrange("p (k m) -> p k m", k=2) for h in range(2)]
            load_super(1)
            mbt = cpool.tile([128, 2 * npair], f32, tag="mbt")
            nc.gpsimd.dma_start(mbt[:], mb_d[:])
            b2ct = cpool.tile([128, 2 * npair], f32, tag="b2ct")
            nc.gpsimd.dma_start(b2ct[:], b2c_d[:])
            outt = cpool.tile([128, 2 * npair], f32, tag="outt")
            sscr = cpool.tile([128, 1024], bf16, tag="sscr")

            w1v = [w1t[h][:].rearrange("p (k m) -> p k m", k=2)
                   for h in range(2)]

            ps1_tiles = {}
            h1_tiles = {}
            ps2_tiles = {}

            def two_region(t, c2):
                # [128, 1024] tile -> [128, 2, c2] AP: region 0 at col 0,
                # region 1 at col 512 (fixed stride keeps matmul outputs
                # within single PSUM banks and DR k-step a multiple of 16)
                return t[:].rearrange("p (k c) -> p k c", k=2)[:, :, 0:c2]

            def emit_l1(i):
                c2 = int(C2[i // 2])
                j = i // SB
                off = int(SUB_OFF[i] - SUPER_OFF[j])
                xv = xts[j % 3][:, off:off + 2 * c2] \
                    .rearrange("p (k c) -> p k c", k=2)
                ps1 = psum1.tile([128, 1024], f32, tag="ps1")
                for h in range(2):
                    nc.tensor.matmul(ps1[:, h * 512:h * 512 + c2],
                                     w1v[h], xv,
                                     start=True, stop=True, perf_mode=DR)
                ps1_tiles[i] = ps1

            def emit_relu1(i):
                c2 = int(C2[i // 2])
                ps1 = ps1_tiles.pop(i)
                h1 = h1pool.tile([128, 1024], fp8, tag="h1")
                nc.scalar.activation(two_region(h1, c2), two_region(ps1, c2),
                                     Act.Relu, scale=1.0 / WSCALE)
                h1_tiles[i] = h1

            def emit_l2(p, h):
                c2 = int(C2[p])
                ps2 = psum2.tile([128, 1024], f32, tag="ps2")
                for b in range(2):
                    h1 = h1_tiles[2 * p + b]
                    h1v = two_region(h1, c2)
                    nc.tensor.matmul(ps2[:, b * 512:b * 512 + c2],
                                     w2hit[h], h1v,
                                     start=True, stop=True, perf_mode=DR)
                ps2_tiles[(p, h)] = ps2
                if h == 1:
                    h1_tiles.pop(2 * p)
                    h1_tiles.pop(2 * p + 1)

            pending_act = []

            def emit_relu2(p, h):
                c = h * npair + p
                # ACT absorbs every ROT-th op, plus the very last (slot, h=1)
                # so the pipeline drain runs on both engines in parallel
                if (2 * p + h) % ROT == ROT - 1 or \
                        (p == npair - 1 and h == 1):
                    pending_act.append((p, h))
                    return
                c2 = int(C2[p])
                ps2 = ps2_tiles.pop((p, h))
                hsc = hscpool.tile([128, 1024], bf16, tag=f"hsc{h}")
                # out = max(ps2, -b2c); accum_out = sum(out)
                #     = segment sum of max(ps2+b2c, 0) minus cap*b2c
                # (host adds the cap*b2c back)
                nc.vector.tensor_scalar(
                    two_region(hsc, c2), two_region(ps2, c2),
                    mbt[:, c:c + 1], 0.0,
                    op0=Alu.max, op1=Alu.add,
                    accum_out=outt[:, c:c + 1])

            def flush_act(upto):
                while pending_act and pending_act[0][0] <= upto:
                    p, h = pending_act.pop(0)
                    c2 = int(C2[p])
                    ps2 = ps2_tiles.pop((p, h))
                    c = h * npair + p
                    hsc = hscpool.tile([128, 1024], bf16, tag=f"hsc{h}")
                    # ACT skips its (expensive, 360ns) accumulator read; the
                    # column sum runs on DVE in fast 16-bit SBUF mode, landing
                    # in the idle hole each ACT absorption creates there
                    nc.scalar.activation(
                        two_region(hsc, c2), two_region(ps2, c2), Act.Relu,
                        bias=b2ct[:, c:c + 1])
                    nc.vector.tensor_scalar(
                        two_region(sscr, c2), two_region(hsc, c2), 1.0, 0.0,
                        op0=Alu.mult, op1=Alu.add,
                        accum_out=outt[:, c:c + 1])

            emit_l1(0)
            emit_relu1(0)
            emit_l1(1)
            emit_relu1(1)
            for p in range(1, npair):
                i0, i1 = 2 * p, 2 * p + 1
                if i0 % SB == 0:
                    load_super(i0 // SB + 1)
                flush_act(p - 2)
                emit_l1(i0)
                emit_relu1(i0)
                emit_l2(p - 1, 0)
                emit_relu2(p - 1, 0)
                emit_l1(i1)
                emit_relu1(i1)
                emit_l2(p - 1, 1)
                emit_relu2(p - 1, 1)
            emit_l2(npair - 1, 0)
            emit_relu2(npair - 1, 0)
            emit_l2(npair - 1, 1)
            emit_relu2(npair - 1, 1)
            flush_act(npair)

            nc.sync.dma_start(out_d[:], outt[:])

    nc.compile()
    return nc


def _get_program(nblk):
    if nblk not in _compiled:
        _compiled[nblk] = _build_program(nblk)
    return _compiled[nblk]


def _q8(x):
    return np.asarray(x, dtype=np.float32).astype(FP8)


def kernel(X, Z, W1, b1, W2, b2, W3, b3, cell_to_batch, sample_idx_batch):
    from scipy.special import erf

    X = np.asarray(X)
    Z = np.asarray(Z, dtype=np.float32)
    W1 = np.asarray(W1, dtype=np.float32)
    b1 = np.asarray(b1, dtype=np.float32)
    W2 = np.asarray(W2, dtype=np.float32)
    b2 = np.asarray(b2, dtype=np.float32)
    W3 = np.asarray(W3, dtype=np.float32)
    b3 = np.asarray(b3, dtype=np.float32)
    c2b = np.asarray(cell_to_batch).astype(np.int64)
    sib = np.asarray(sample_idx_batch).astype(np.int64)

    n = X.shape[0]
    nseg = sib.shape[0]
    seg = sib[c2b]
    npair = NSLOT
    assert nseg == N_CORES * NSLOT

    # ---- host layout prep -------------------------------------------------
    counts = np.bincount(seg, minlength=nseg).astype(np.int64)
    # per-core slot assignment: sort the core's segments by count descending
    seg2slot = np.zeros(nseg, dtype=np.int64)      # segment -> local slot
    slot2seg = np.zeros((N_CORES, NSLOT), dtype=np.int64)
    for c in range(N_CORES):
        segs = np.arange(c * NSLOT, (c + 1) * NSLOT)
        order_c = np.argsort(-counts[segs], kind="stable")
        slot2seg[c] = segs[order_c]
        seg2slot[segs[order_c]] = np.arange(NSLOT)
    seg_cap = CAPS[seg2slot]                       # [nseg] device capacity

    order = np.argsort(seg, kind="stable")
    seg_sorted = seg[order]
    run_starts = np.concatenate([[0], np.cumsum(counts)])[:nseg]
    ranks = np.arange(n, dtype=np.int64) - run_starts[seg_sorted]
    on_dev = ranks < seg_cap[seg_sorted]

    # column offset of each segment's data region inside its core's stream:
    # slot q data cols for sub-block b start at SUB_OFF[2q + b]
    xl8 = _q8(np.log1p(np.asarray(X, dtype=np.float32)))
    xt = np.zeros((N_CORES, DX, TOTCOL), dtype=FP8)
    one8 = np.float32(1.0).astype(FP8)
    c2_of_seg = (seg_cap // 2)
    # destination column for each on-device cell
    q_of_seg = seg2slot
    sub0 = SUB_OFF[2 * q_of_seg[seg_sorted]]       # per cell (via its seg)
    sub1 = SUB_OFF[2 * q_of_seg[seg_sorted] + 1]
    c2s = c2_of_seg[seg_sorted]
    in_b1 = ranks >= c2s
    col = np.where(in_b1, sub1 + (ranks - c2s), sub0 + ranks)
    core_of = seg_sorted // NSLOT
    od = on_dev
    xt[core_of[od], :, col[od]] = xl8[order[od]]
    # one-hot columns: for every slot/sub-block, cols [c2:2*c2] rows 2q,2q+1
    for q in range(NSLOT):
        c2q = int(C2[q])
        for b in range(2):
            o = int(SUB_OFF[2 * q + b])
            xt[:, 2 * q, o + c2q:o + 2 * c2q] = one8
            xt[:, 2 * q + 1, o + c2q:o + 2 * c2q] = one8

    # ---- weights ----------------------------------------------------------
    W1x = W1[:DX]
    w1_hi = _q8(W1x * WSCALE)
    w1dev = w1_hi.astype(np.float32) / WSCALE
    dW1 = w1dev - W1x
    w2_hi = _q8(W2 * WSCALE)
    dW2 = w2_hi.astype(np.float32) / WSCALE - W2

    w2hiq = np.zeros((128, 512), dtype=FP8)
    for h in range(2):
        for t in range(2):
            w2hiq[:, h * 256 + t * 128:h * 256 + (t + 1) * 128] = \
                w2_hi[t * 128:(t + 1) * 128, h * 128:(h + 1) * 128]

    # zb with the W1-quantization mean-correction, as fp8 hi+lo table rows
    xlf = xl8.astype(np.float32)
    mcol = xlf.mean(axis=0)
    vcol = xlf.var(axis=0)
    zb = Z @ W1[DX:DX + DZ] + b1 - mcol @ dW1    # [B, 256]
    zbs = zb * WSCALE
    zb_hi = _q8(zbs)
    zb_lo = _q8(zbs - zb_hi.astype(np.float32))
    zbq = (zb_hi.astype(np.float32) + zb_lo.astype(np.float32)) / WSCALE

    # per-core w1: [h][128, 256] = [64*W1x_hi | zb table]
    w1q = np.zeros((N_CORES, 2, 128, 256), dtype=FP8)
    for c in range(N_CORES):
        for h in range(2):
            w1q[c, h, :, 0:128] = w1_hi[:, h * 128:(h + 1) * 128]
            tab = np.zeros((128, 128), dtype=FP8)
            tab[2 * np.arange(NSLOT)] = zb_hi[slot2seg[c], h * 128:(h + 1) * 128]
            tab[2 * np.arange(NSLOT) + 1] = zb_lo[slot2seg[c], h * 128:(h + 1) * 128]
            w1q[c, h, :, 128:256] = tab

    # E[h1|seg] Gaussian closed form -> per-segment W2 mean-correction
    mu = mcol @ w1dev
    sig = np.sqrt(np.maximum(vcol @ (w1dev ** 2), 1e-12))
    muz = mu[None, :] + zbq
    u = muz / sig[None, :]
    Phi = 0.5 * (1.0 + erf(u / np.sqrt(2.0)))
    phi = np.exp(-0.5 * u * u) / np.sqrt(2.0 * np.pi)
    Eh1 = sig[None, :] * phi + muz * Phi
    b2c_seg = WSCALE * (b2[None, :] - Eh1 @ dW2)   # [B, 256]

    def per_slot_cols(src):                      # [B, 256] -> [core, 128, 2np]
        pv = src[slot2seg.reshape(-1)]           # core-major slot order
        a = pv.reshape(N_CORES, npair, 2, 128).transpose(0, 3, 2, 1)
        return np.ascontiguousarray(a).reshape(N_CORES, 128, 2 * npair)

    b2cd = per_slot_cols(b2c_seg)
    mbd = per_slot_cols(-b2c_seg)

    # ---- run on 8 cores ---------------------------------------------------
    nc = _get_program(NBLK)
    in_maps = []
    for c in range(N_CORES):
        in_maps.append({
            "xt": xt[c], "w1": w1q[c], "w2hi": w2hiq,
            "mb": mbd[c], "b2c": b2cd[c],
        })
    global _last_in_maps
    _last_in_maps = in_maps
    res = run_bass_kernel_spmd(nc, in_maps, list(range(N_CORES)))

    # ---- host epilogue ----------------------------------------------------
    per_core = []
    for c in range(N_CORES):
        o = res.results[c]["out"]               # [128, 2*npair]
        per_core.append(np.stack([o[:, 0:npair], o[:, npair:2 * npair]],
                                 axis=0))
    sums_slot = np.concatenate(per_core, axis=2)   # [2, 128, ncore*nslot]
    sums_slot = sums_slot.transpose(2, 0, 1).reshape(nseg, H)  # slot-major

    # map slot-major rows back to segment ids
    seg_of_row = slot2seg.reshape(-1)              # row r -> segment
    sums = np.zeros((nseg, H), dtype=np.float32)
    sums[seg_of_row] = sums_slot

    # DVE-computed (slot, half) columns used the shift identity: add the
    # cap*b2c they are missing.  ACT-computed ones are already complete.
    lq = seg2slot                                  # local slot of each segment
    dve_mask = np.stack(
        [((2 * lq + h) % ROT != ROT - 1) & ~((lq == npair - 1) & (h == 1))
         for h in range(2)], axis=1)               # [nseg, 2]
    shift = np.repeat(dve_mask, 128, axis=1) * \
        (seg_cap[:, None].astype(np.float32) * b2c_seg)
    sums = sums + shift

    # pad-cell contribution: xl = 0 -> h1 = fp8(relu(zbq))
    h1p = _q8(np.maximum(zbq, 0.0)).astype(np.float32)         # [B, 256]
    pre = h1p @ w2_hi.astype(np.float32) + b2c_seg
    h2p = np.maximum(pre, 0.0)                                 # [B, 256]

    real = np.minimum(counts, seg_cap)
    corr = (seg_cap - real).astype(np.float32)[:, None] * h2p

    S = (sums - corr) / WSCALE

    # host-evaluated overflow cells (exact f32 math)
    idx_host = order[~on_dev]
    if idx_host.shape[0]:
        hh1 = np.maximum(np.log1p(X[idx_host].astype(np.float32)) @ W1x
                         + (Z @ W1[DX:DX + DZ] + b1)[seg[idx_host]], 0.0)
        hh2 = np.maximum(hh1 @ W2 + b2, 0.0)
        np.add.at(S, seg[idx_host], hh2)

    denom = np.maximum(counts, 1).astype(np.float32)[:, None]
    Y = S @ W3 / denom + b3[None, :]
    Y[counts == 0] = 0.0
    return Y.astype(np.float32)


# revision 43
# speedup vs baseline: 1.0492x; 1.0492x over previous
"""Trainium2 Bass kernel for CompositionModel (gnn_message_passing).

Model: per-cell MLP over [log1p(X) ++ Z[cell_to_batch]] followed by a
segment-mean over batch labels.

Strategy (v6 — one-hot segment table + variable slot capacities):
  * Each core owns 64 segments, sorted by cell count into 64 SLOTS with
    compiled capacities matched to the order statistics of the multinomial
    count distribution (caps rounded to 32, max 1024).  Overflow cells are
    evaluated exactly on the host.
  * Cell stream per sub-block interleaves [xl fp8 | one-hot fp8] columns:
    the one-hot column has ones at rows (2q, 2q+1) selecting slot q's
    zb hi/lo rows from a STATIC per-core weight table, so the L1 DoubleRow
    matmul adds the per-segment bias zb = Z @ W1z + b1 with no per-pair
    weight rewriting (no GPSIMD copies, no tile rotation).  W1 quantization
    error is mean-corrected through zb.
  * ACT: relu1 per sub-block (scale=1/64) -> fp8 h1.
  * L2: per (slot, half) two DoubleRow matmuls -> one fp32 PSUM tile;
    quantization mean-corrected per segment via b2c.
  * relu2 + segment sum: ONE op per (slot, half) with accum_out.  DVE path
    uses max(x+b,0) = max(x,-b)+b (single-source tensor_scalar, accumulator
    op add); host adds cap*b back.  Every ROT-th op runs on ACT (Relu +
    bias + accum) for engine balance, flushed one slot late.
  * The third (linear) layer commutes with the segment sum and is applied
    on the host; pad-cell contributions are subtracted analytically.
"""

import numpy as np
import ml_dtypes

import concourse.bacc as bacc
import concourse.mybir as mybir
import concourse.tile as tile
from concourse.bass_utils import run_bass_kernel_spmd

BF16 = ml_dtypes.bfloat16
FP8 = ml_dtypes.float8_e4m3fn

N_CORES = 8
DX = 128
DZ = 32
H = 256
B = 512
N_CELLS = 500_000
NSLOT = 64         # segments (slots) per core
SB = 4             # sub-blocks per superblock (one DMA)
NBLK = 2 * NSLOT   # sub-blocks per core
WSCALE = 64.0      # fp8 pre-scale on W1/W2/zb/b2, divided out at the end
ROT = 12           # every ROT-th relu2 op runs on ACT instead of DVE

_compiled = {}
_last_in_maps = None


def _slot_caps():
    """Compiled per-slot capacities: Blom-approximated expected order
    statistics (descending) of a core's 64 segment counts, rounded up to
    32, clamped to [64, 1024]."""
    from scipy.special import ndtri
    p = 1.0 / B
    mu = N_CELLS * p
    sig = np.sqrt(N_CELLS * p * (1 - p))
    q = np.arange(NSLOT, dtype=np.float64)          # 0 = largest
    r = NSLOT - q                                    # rank from smallest
    z = ndtri((r - 0.375) / (NSLOT + 0.25))
    caps = mu + sig * z
    caps = (np.round(caps / 32.0) * 32).astype(np.int64)
    return np.clip(caps, 64, 1024)


CAPS = _slot_caps()                    # [NSLOT] descending
C2 = CAPS // 2                         # per-sub-block cell count
SUB_W = 2 * C2                         # stream cols per sub-block (xl+onehot)
# sub-block i belongs to slot i//2; superblock j covers sub-blocks 4j..4j+3
SUB_OFF = np.concatenate([[0], np.cumsum(np.repeat(SUB_W, 2))])  # [NBLK+1]
TOTCOL = int(SUB_OFF[-1])
NSUPER = NBLK // SB
SUPER_OFF = SUB_OFF[::SB]              # [NSUPER+1]


def _build_program(nblk):
    f32 = mybir.dt.float32
    bf16 = mybir.dt.bfloat16
    fp8 = mybir.dt.float8e4
    Alu = mybir.AluOpType
    Act = mybir.ActivationFunctionType
    DR = mybir.MatmulPerfMode.DoubleRow
    assert nblk == NBLK
    npair = NSLOT

    nc = bacc.Bacc("TRN2", target_bir_lowering=False, debug=False,
                   num_devices=N_CORES)

    xt_d = nc.dram_tensor("xt", [DX, TOTCOL], fp8, kind="ExternalInput")
    # per half: [128, 256] = [64*W1x_hi | zb segment table (rows 2q,2q+1)]
    w1_d = nc.dram_tensor("w1", [2, 128, 256], fp8, kind="ExternalInput")
    w2hi_d = nc.dram_tensor("w2hi", [128, 512], fp8, kind="ExternalInput")
    mb_d = nc.dram_tensor("mb", [128, 2 * npair], f32, kind="ExternalInput")
    b2c_d = nc.dram_tensor("b2c", [128, 2 * npair], f32, kind="ExternalInput")
    out_d = nc.dram_tensor("out", [128, 2 * npair], f32, kind="ExternalOutput")

    with tile.TileContext(nc) as tc:
        with tc.tile_pool(name="consts", bufs=1) as cpool, \
             tc.tile_pool(name="h1", bufs=4) as h1pool, \
             tc.tile_pool(name="hsc", bufs=2) as hscpool, \
             tc.tile_pool(name="ps1", bufs=2, space="PSUM") as psum1, \
             tc.tile_pool(name="ps2", bufs=2, space="PSUM") as psum2:

            # startup DMAs: critical chain (xt0, w1) on the Sync queue;
            # the rest triggered from the idle GpSimd queue in parallel
            xts = [cpool.tile([DX, 4096], fp8, tag=f"xt{r}", name=f"xt{r}")
                   for r in range(3)]

            def load_super(j):
                if j >= NSUPER:
                    return
                o0, o1 = int(SUPER_OFF[j]), int(SUPER_OFF[j + 1])
                nc.sync.dma_start(xts[j % 3][:, 0:o1 - o0], xt_d[:, o0:o1])

            # w1 first (gates the first LDWEIGHTS), then superblock 0 split
            # across two triggers so its transfer spreads over two queues
            w1t = []
            for h in range(2):
                w = cpool.tile([128, 256], fp8, tag=f"w1_{h}",
                               name=f"w1_{h}")
                nc.sync.dma_start(w[:], w1_d[h])
                w1t.append(w)
            s0, s1 = int(SUPER_OFF[0]), int(SUPER_OFF[1])
            smid = s0 + ((s1 - s0) // 2 // 16) * 16
            nc.sync.dma_start(xts[0][:, 0:smid - s0], xt_d[:, s0:smid])
            nc.sync.dma_start(xts[0][:, smid - s0:s1 - s0], xt_d[:, smid:s1])
            w2 = cpool.tile([128, 512], fp8, tag="w2")
            nc.gpsimd.dma_start(w2[:], w2hi_d[:])
            w2hit = [w2[:, h * 256:(h + 1) * 256]
                     .rearrange("p (k m) -> p k m", k=2) for h in range(2)]
            load_super(1)
            mbt = cpool.tile([128, 2 * npair], f32, tag="mbt")
            nc.gpsimd.dma_start(mbt[:], mb_d[:])
            b2ct = cpool.tile([128, 2 * npair], f32, tag="b2ct")
            nc.gpsimd.dma_start(b2ct[:], b2c_d[:])
            outt = cpool.tile([128, 2 * npair], f32, tag="outt")

            w1v = [w1t[h][:].rearrange("p (k m) -> p k m", k=2)
                   for h in range(2)]

            ps1_tiles = {}
            h1_tiles = {}
            ps2_tiles = {}

            def two_region(t, c2):
                # [128, 1024] tile -> [128, 2, c2] AP: region 0 at col 0,
                # region 1 at col 512 (fixed stride keeps matmul outputs
                # within single PSUM banks and DR k-step a multiple of 16)
                return t[:].rearrange("p (k c) -> p k c", k=2)[:, :, 0:c2]

            def emit_l1(i):
                c2 = int(C2[i // 2])
                j = i // SB
                off = int(SUB_OFF[i] - SUPER_OFF[j])
                xv = xts[j % 3][:, off:off + 2 * c2] \
                    .rearrange("p (k c) -> p k c", k=2)
                ps1 = psum1.tile([128, 1024], f32, tag="ps1")
                for h in range(2):
                    nc.tensor.matmul(ps1[:, h * 512:h * 512 + c2],
                                     w1v[h], xv,
                                     start=True, stop=True, perf_mode=DR)
                ps1_tiles[i] = ps1

            def emit_relu1(i):
                c2 = int(C2[i // 2])
                ps1 = ps1_tiles.pop(i)
                h1 = h1pool.tile([128, 1024], fp8, tag="h1")
                nc.scalar.activation(two_region(h1, c2), two_region(ps1, c2),
                                     Act.Relu, scale=1.0 / WSCALE)
                h1_tiles[i] = h1

            def emit_l2(p, h):
                c2 = int(C2[p])
                ps2 = psum2.tile([128, 1024], f32, tag="ps2")
                for b in range(2):
                    h1 = h1_tiles[2 * p + b]
                    h1v = two_region(h1, c2)
                    nc.tensor.matmul(ps2[:, b * 512:b * 512 + c2],
                                     w2hit[h], h1v,
                                     start=True, stop=True, perf_mode=DR)
                ps2_tiles[(p, h)] = ps2
                if h == 1:
                    h1_tiles.pop(2 * p)
                    h1_tiles.pop(2 * p + 1)

            pending_act = []

            def emit_relu2(p, h):
                c = h * npair + p
                # ACT absorbs every ROT-th op, plus the very last (slot, h=1)
                # so the pipeline drain runs on both engines in parallel
                if (2 * p + h) % ROT == ROT - 1 or \
                        (p == npair - 1 and h == 1):
                    pending_act.append((p, h))
                    return
                c2 = int(C2[p])
                ps2 = ps2_tiles.pop((p, h))
                hsc = hscpool.tile([128, 1024], bf16, tag=f"hsc{h}")
                # out = max(ps2, -b2c); accum_out = sum(out)
                #     = segment sum of max(ps2+b2c, 0) minus cap*b2c
                # (host adds the cap*b2c back)
                nc.vector.tensor_scalar(
                    two_region(hsc, c2), two_region(ps2, c2),
                    mbt[:, c:c + 1], 0.0,
                    op0=Alu.max, op1=Alu.add,
                    accum_out=outt[:, c:c + 1])

            def flush_act(upto):
                while pending_act and pending_act[0][0] <= upto:
                    p, h = pending_act.pop(0)
                    c2 = int(C2[p])
                    ps2 = ps2_tiles.pop((p, h))
                    c = h * npair + p
                    hsc = hscpool.tile([128, 1024], bf16, tag=f"hsc{h}")
                    nc.scalar.activation(
                        two_region(hsc, c2), two_region(ps2, c2), Act.Relu,
                        bias=b2ct[:, c:c + 1], accum_out=outt[:, c:c + 1])

            emit_l1(0)
            emit_relu1(0)
            emit_l1(1)
            emit_relu1(1)
            for p in range(1, npair):
                i0, i1 = 2 * p, 2 * p + 1
                if i0 % SB == 0:
                    load_super(i0 // SB + 1)
                flush_act(p - 2)
                emit_l1(i0)
                emit_relu1(i0)
                emit_l2(p - 1, 0)
                emit_relu2(p - 1, 0)
                emit_l1(i1)
                emit_relu1(i1)
                emit_l2(p - 1, 1)
                emit_relu2(p - 1, 1)
            emit_l2(npair - 1, 0)
            emit_relu2(npair - 1, 0)
            emit_l2(npair - 1, 1)
            emit_relu2(npair - 1, 1)
            flush_act(npair)

            nc.sync.dma_start(out_d[:], outt[:])

    nc.compile()
    return nc


def _get_program(nblk):
    if nblk not in _compiled:
        _compiled[nblk] = _build_program(nblk)
    return _compiled[nblk]


def _q8(x):
    return np.asarray(x, dtype=np.float32).astype(FP8)


def kernel(X, Z, W1, b1, W2, b2, W3, b3, cell_to_batch, sample_idx_batch):
    from scipy.special import erf

    X = np.asarray(X)
    Z = np.asarray(Z, dtype=np.float32)
    W1 = np.asarray(W1, dtype=np.float32)
    b1 = np.asarray(b1, dtype=np.float32)
    W2 = np.asarray(W2, dtype=np.float32)
    b2 = np.asarray(b2, dtype=np.float32)
    W3 = np.asarray(W3, dtype=np.float32)
    b3 = np.asarray(b3, dtype=np.float32)
    c2b = np.asarray(cell_to_batch).astype(np.int64)
    sib = np.asarray(sample_idx_batch).astype(np.int64)

    n = X.shape[0]
    nseg = sib.shape[0]
    seg = sib[c2b]
    npair = NSLOT
    assert nseg == N_CORES * NSLOT

    # ---- host layout prep -------------------------------------------------
    counts = np.bincount(seg, minlength=nseg).astype(np.int64)
    # per-core slot assignment: sort the core's segments by count descending
    seg2slot = np.zeros(nseg, dtype=np.int64)      # segment -> local slot
    slot2seg = np.zeros((N_CORES, NSLOT), dtype=np.int64)
    for c in range(N_CORES):
        segs = np.arange(c * NSLOT, (c + 1) * NSLOT)
        order_c = np.argsort(-counts[segs], kind="stable")
        slot2seg[c] = segs[order_c]
        seg2slot[segs[order_c]] = np.arange(NSLOT)
    seg_cap = CAPS[seg2slot]                       # [nseg] device capacity

    order = np.argsort(seg, kind="stable")
    seg_sorted = seg[order]
    run_starts = np.concatenate([[0], np.cumsum(counts)])[:nseg]
    ranks = np.arange(n, dtype=np.int64) - run_starts[seg_sorted]
    on_dev = ranks < seg_cap[seg_sorted]

    # column offset of each segment's data region inside its core's stream:
    # slot q data cols for sub-block b start at SUB_OFF[2q + b]
    xl8 = _q8(np.log1p(np.asarray(X, dtype=np.float32)))
    xt = np.zeros((N_CORES, DX, TOTCOL), dtype=FP8)
    one8 = np.float32(1.0).astype(FP8)
    c2_of_seg = (seg_cap // 2)
    # destination column for each on-device cell
    q_of_seg = seg2slot
    sub0 = SUB_OFF[2 * q_of_seg[seg_sorted]]       # per cell (via its seg)
    sub1 = SUB_OFF[2 * q_of_seg[seg_sorted] + 1]
    c2s = c2_of_seg[seg_sorted]
    in_b1 = ranks >= c2s
    col = np.where(in_b1, sub1 + (ranks - c2s), sub0 + ranks)
    core_of = seg_sorted // NSLOT
    od = on_dev
    xt[core_of[od], :, col[od]] = xl8[order[od]]
    # one-hot columns: for every slot/sub-block, cols [c2:2*c2] rows 2q,2q+1
    for q in range(NSLOT):
        c2q = int(C2[q])
        for b in range(2):
            o = int(SUB_OFF[2 * q + b])
            xt[:, 2 * q, o + c2q:o + 2 * c2q] = one8
            xt[:, 2 * q + 1, o + c2q:o + 2 * c2q] = one8

    # ---- weights ----------------------------------------------------------
    W1x = W1[:DX]
    w1_hi = _q8(W1x * WSCALE)
    w1dev = w1_hi.astype(np.float32) / WSCALE
    dW1 = w1dev - W1x
    w2_hi = _q8(W2 * WSCALE)
    dW2 = w2_hi.astype(np.float32) / WSCALE - W2

    w2hiq = np.zeros((128, 512), dtype=FP8)
    for h in range(2):
        for t in range(2):
            w2hiq[:, h * 256 + t * 128:h * 256 + (t + 1) * 128] = \
                w2_hi[t * 128:(t + 1) * 128, h * 128:(h + 1) * 128]

    # zb with the W1-quantization mean-correction, as fp8 hi+lo table rows
    xlf = xl8.astype(np.float32)
    mcol = xlf.mean(axis=0)
    vcol = xlf.var(axis=0)
    zb = Z @ W1[DX:DX + DZ] + b1 - mcol @ dW1    # [B, 256]
    zbs = zb * WSCALE
    zb_hi = _q8(zbs)
    zb_lo = _q8(zbs - zb_hi.astype(np.float32))
    zbq = (zb_hi.astype(np.float32) + zb_lo.astype(np.float32)) / WSCALE

    # per-core w1: [h][128, 256] = [64*W1x_hi | zb table]
    w1q = np.zeros((N_CORES, 2, 128, 256), dtype=FP8)
    for c in range(N_CORES):
        for h in range(2):
            w1q[c, h, :, 0:128] = w1_hi[:, h * 128:(h + 1) * 128]
            tab = np.zeros((128, 128), dtype=FP8)
            tab[2 * np.arange(NSLOT)] = zb_hi[slot2seg[c], h * 128:(h + 1) * 128]
            tab[2 * np.arange(NSLOT) + 1] = zb_lo[slot2seg[c], h * 128:(h + 1) * 128]
            w1q[c, h, :, 128:256] = tab

    # E[h1|seg] Gaussian closed form -> per-segment W2 mean-correction
    mu = mcol @ w1dev
    sig = np.sqrt(np.maximum(vcol @ (w1dev ** 2), 1e-12))
    muz = mu[None, :] + zbq
    u = muz / sig[None, :]
    Phi = 0.5 * (1.0 + erf(u / np.sqrt(2.0)))
    phi = np.exp(-0.5 * u * u) / np.sqrt(2.0 * np.pi)
    Eh1 = sig[None, :] * phi + muz * Phi
    b2c_seg = WSCALE * (b2[None, :] - Eh1 @ dW2)   # [B, 256]

    def per_slot_cols(src):                      # [B, 256] -> [core, 128, 2np]
        pv = src[slot2seg.reshape(-1)]           # core-major slot order
        a = pv.reshape(N_CORES, npair, 2, 128).transpose(0, 3, 2, 1)
        return np.ascontiguousarray(a).reshape(N_CORES, 128, 2 * npair)

    b2cd = per_slot_cols(b2c_seg)
    mbd = per_slot_cols(-b2c_seg)

    # ---- run on 8 cores ---------------------------------------------------
    nc = _get_program(NBLK)
    in_maps = []
    for c in range(N_CORES):
        in_maps.append({
            "xt": xt[c], "w1": w1q[c], "w2hi": w2hiq,
            "mb": mbd[c], "b2c": b2cd[c],
        })
    global _last_in_maps
    _last_in_maps = in_maps
    res = run_bass_kernel_spmd(nc, in_maps, list(range(N_CORES)))

    # ---- host epilogue ----------------------------------------------------
    per_core = []
    for c in range(N_CORES):
        o = res.results[c]["out"]               # [128, 2*npair]
        per_core.append(np.stack([o[:, 0:npair], o[:, npair:2 * npair]],
                                 axis=0))
    sums_slot = np.concatenate(per_core, axis=2)   # [2, 128, ncore*nslot]
    sums_slot = sums_slot.transpose(2, 0, 1).reshape(nseg, H)  # slot-major

    # map slot-major rows back to segment ids
    seg_of_row = slot2seg.reshape(-1)              # row r -> segment
    sums = np.zeros((nseg, H), dtype=np.float32)
    sums[seg_of_row] = sums_slot

    # DVE-computed (slot, half) columns used the shift identity: add the
    # cap*b2c they are missing.  ACT-computed ones are already complete.
    lq = seg2slot                                  # local slot of each segment
    dve_mask = np.stack(
        [((2 * lq + h) % ROT != ROT - 1) & ~((lq == npair - 1) & (h == 1))
         for h in range(2)], axis=1)               # [nseg, 2]
    shift = np.repeat(dve_mask, 128, axis=1) * \
        (seg_cap[:, None].astype(np.float32) * b2c_seg)
    sums = sums + shift

    # pad-cell contribution: xl = 0 -> h1 = fp8(relu(zbq))
    h1p = _q8(np.maximum(zbq, 0.0)).astype(np.float32)         # [B, 256]
    pre = h1p @ w2_hi.astype(np.float32) + b2c_seg
    h2p = np.maximum(pre, 0.0)                                 # [B, 256]

    real = np.minimum(counts, seg_cap)
    corr = (seg_cap - real).astype(np.float32)[:, None] * h2p

    S = (sums - corr) / WSCALE

    # host-evaluated overflow cells (exact f32 math)
    idx_host = order[~on_dev]
    if idx_host.shape[0]:
        hh1 = np.maximum(np.log1p(X[idx_host].astype(np.float32)) @ W1x
                         + (Z @ W1[DX:DX + DZ] + b1)[seg[idx_host]], 0.0)
        hh2 = np.maximum(hh1 @ W2 + b2, 0.0)
        np.add.at(S, seg[idx_host], hh2)

    denom = np.maximum(counts, 1).astype(np.float32)[:, None]
    Y = S @ W3 / denom + b3[None, :]
    Y[counts == 0] = 0.0
    return Y.astype(np.float32)


# revision 44
# speedup vs baseline: 1.0675x; 1.0174x over previous
"""Trainium2 Bass kernel for CompositionModel (gnn_message_passing).

Model: per-cell MLP over [log1p(X) ++ Z[cell_to_batch]] followed by a
segment-mean over batch labels.

Strategy (v6 — one-hot segment table + variable slot capacities):
  * Each core owns 64 segments, sorted by cell count into 64 SLOTS with
    compiled capacities matched to the order statistics of the multinomial
    count distribution (caps rounded to 32, max 1024).  Overflow cells are
    evaluated exactly on the host.
  * Cell stream per sub-block interleaves [xl fp8 | one-hot fp8] columns:
    the one-hot column has ones at rows (2q, 2q+1) selecting slot q's
    zb hi/lo rows from a STATIC per-core weight table, so the L1 DoubleRow
    matmul adds the per-segment bias zb = Z @ W1z + b1 with no per-pair
    weight rewriting (no GPSIMD copies, no tile rotation).  W1 quantization
    error is mean-corrected through zb.
  * ACT: relu1 per sub-block (scale=1/64) -> fp8 h1.
  * L2: per (slot, half) two DoubleRow matmuls -> one fp32 PSUM tile;
    quantization mean-corrected per segment via b2c.
  * relu2 + segment sum: ONE op per (slot, half) with accum_out.  DVE path
    uses max(x+b,0) = max(x,-b)+b (single-source tensor_scalar, accumulator
    op add); host adds cap*b back.  Every ROT-th op runs on ACT (Relu +
    bias + accum) for engine balance, flushed one slot late.
  * The third (linear) layer commutes with the segment sum and is applied
    on the host; pad-cell contributions are subtracted analytically.
"""

import numpy as np
import ml_dtypes

import concourse.bacc as bacc
import concourse.mybir as mybir
import concourse.tile as tile
from concourse.bass_utils import run_bass_kernel_spmd

BF16 = ml_dtypes.bfloat16
FP8 = ml_dtypes.float8_e4m3fn

N_CORES = 8
DX = 128
DZ = 32
H = 256
B = 512
N_CELLS = 500_000
NSLOT = 64         # segments (slots) per core
SB = 4             # sub-blocks per superblock (one DMA)
NBLK = 2 * NSLOT   # sub-blocks per core
WSCALE = 64.0      # fp8 pre-scale on W1/W2/zb/b2, divided out at the end
ROT = 12           # every ROT-th relu2 op runs on ACT instead of DVE

_compiled = {}
_last_in_maps = None


def _slot_caps():
    """Compiled per-slot capacities: Blom-approximated expected order
    statistics (descending) of a core's 64 segment counts, rounded up to
    32, clamped to [64, 1024]."""
    from scipy.special import ndtri
    p = 1.0 / B
    mu = N_CELLS * p
    sig = np.sqrt(N_CELLS * p * (1 - p))
    q = np.arange(NSLOT, dtype=np.float64)          # 0 = largest
    r = NSLOT - q                                    # rank from smallest
    z = ndtri((r - 0.375) / (NSLOT + 0.25))
    caps = mu + sig * z
    caps = (np.round(caps / 32.0) * 32).astype(np.int64)
    return np.clip(caps, 64, 1024)


CAPS = _slot_caps()                    # [NSLOT] descending
C2 = CAPS // 2                         # per-sub-block cell count
SUB_W = 2 * C2                         # stream cols per sub-block (xl+onehot)
# sub-block i belongs to slot i//2; superblock j covers sub-blocks 4j..4j+3
SUB_OFF = np.concatenate([[0], np.cumsum(np.repeat(SUB_W, 2))])  # [NBLK+1]
TOTCOL = int(SUB_OFF[-1])
NSUPER = NBLK // SB
SUPER_OFF = SUB_OFF[::SB]              # [NSUPER+1]


def _build_program(nblk):
    f32 = mybir.dt.float32
    bf16 = mybir.dt.bfloat16
    fp8 = mybir.dt.float8e4
    Alu = mybir.AluOpType
    Act = mybir.ActivationFunctionType
    DR = mybir.MatmulPerfMode.DoubleRow
    assert nblk == NBLK
    npair = NSLOT

    nc = bacc.Bacc("TRN2", target_bir_lowering=False, debug=False,
                   num_devices=N_CORES)

    xt_d = nc.dram_tensor("xt", [DX, TOTCOL], fp8, kind="ExternalInput")
    # per half: [128, 256] = [64*W1x_hi | zb segment table (rows 2q,2q+1)]
    w1_d = nc.dram_tensor("w1", [2, 128, 256], fp8, kind="ExternalInput")
    w2hi_d = nc.dram_tensor("w2hi", [128, 512], fp8, kind="ExternalInput")
    mb_d = nc.dram_tensor("mb", [128, 2 * npair], f32, kind="ExternalInput")
    b2c_d = nc.dram_tensor("b2c", [128, 2 * npair], f32, kind="ExternalInput")
    out_d = nc.dram_tensor("out", [128, 2 * npair], f32, kind="ExternalOutput")

    with tile.TileContext(nc) as tc:
        with tc.tile_pool(name="consts", bufs=1) as cpool, \
             tc.tile_pool(name="h1", bufs=6) as h1pool, \
             tc.tile_pool(name="hsc", bufs=3) as hscpool, \
             tc.tile_pool(name="ps1", bufs=2, space="PSUM") as psum1, \
             tc.tile_pool(name="ps2", bufs=2, space="PSUM") as psum2:

            # startup DMAs: critical chain (xt0, w1) on the Sync queue;
            # the rest triggered from the idle GpSimd queue in parallel
            xts = [cpool.tile([DX, 4096], fp8, tag=f"xt{r}", name=f"xt{r}")
                   for r in range(3)]

            def load_super(j):
                if j >= NSUPER:
                    return
                o0, o1 = int(SUPER_OFF[j]), int(SUPER_OFF[j + 1])
                nc.sync.dma_start(xts[j % 3][:, 0:o1 - o0], xt_d[:, o0:o1])

            # w1 first (gates the first LDWEIGHTS), then superblock 0 split
            # across two triggers so its transfer spreads over two queues
            w1t = []
            for h in range(2):
                w = cpool.tile([128, 256], fp8, tag=f"w1_{h}",
                               name=f"w1_{h}")
                nc.sync.dma_start(w[:], w1_d[h])
                w1t.append(w)
            s0, s1 = int(SUPER_OFF[0]), int(SUPER_OFF[1])
            smid = s0 + ((s1 - s0) // 2 // 16) * 16
            nc.sync.dma_start(xts[0][:, 0:smid - s0], xt_d[:, s0:smid])
            nc.sync.dma_start(xts[0][:, smid - s0:s1 - s0], xt_d[:, smid:s1])
            w2 = cpool.tile([128, 512], fp8, tag="w2")
            nc.gpsimd.dma_start(w2[:], w2hi_d[:])
            w2hit = [w2[:, h * 256:(h + 1) * 256]
                     .rearrange("p (k m) -> p k m", k=2) for h in range(2)]
            load_super(1)
            mbt = cpool.tile([128, 2 * npair], f32, tag="mbt")
            nc.gpsimd.dma_start(mbt[:], mb_d[:])
            b2ct = cpool.tile([128, 2 * npair], f32, tag="b2ct")
            nc.gpsimd.dma_start(b2ct[:], b2c_d[:])
            outt = cpool.tile([128, 2 * npair], f32, tag="outt")

            w1v = [w1t[h][:].rearrange("p (k m) -> p k m", k=2)
                   for h in range(2)]

            ps1_tiles = {}
            h1_tiles = {}
            ps2_tiles = {}

            def two_region(t, c2):
                # [128, 1024] tile -> [128, 2, c2] AP: region 0 at col 0,
                # region 1 at col 512 (fixed stride keeps matmul outputs
                # within single PSUM banks and DR k-step a multiple of 16)
                return t[:].rearrange("p (k c) -> p k c", k=2)[:, :, 0:c2]

            def emit_l1(i):
                c2 = int(C2[i // 2])
                j = i // SB
                off = int(SUB_OFF[i] - SUPER_OFF[j])
                xv = xts[j % 3][:, off:off + 2 * c2] \
                    .rearrange("p (k c) -> p k c", k=2)
                ps1 = psum1.tile([128, 1024], f32, tag="ps1")
                for h in range(2):
                    nc.tensor.matmul(ps1[:, h * 512:h * 512 + c2],
                                     w1v[h], xv,
                                     start=True, stop=True, perf_mode=DR)
                ps1_tiles[i] = ps1

            def emit_relu1(i):
                c2 = int(C2[i // 2])
                ps1 = ps1_tiles.pop(i)
                h1 = h1pool.tile([128, 1024], fp8, tag="h1")
                nc.scalar.activation(two_region(h1, c2), two_region(ps1, c2),
                                     Act.Relu, scale=1.0 / WSCALE)
                h1_tiles[i] = h1

            def emit_l2(p, h):
                c2 = int(C2[p])
                ps2 = psum2.tile([128, 1024], f32, tag="ps2")
                for b in range(2):
                    h1 = h1_tiles[2 * p + b]
                    h1v = two_region(h1, c2)
                    nc.tensor.matmul(ps2[:, b * 512:b * 512 + c2],
                                     w2hit[h], h1v,
                                     start=True, stop=True, perf_mode=DR)
                ps2_tiles[(p, h)] = ps2
                if h == 1:
                    h1_tiles.pop(2 * p)
                    h1_tiles.pop(2 * p + 1)

            pending_act = []

            def emit_relu2(p, h):
                c = h * npair + p
                # ACT absorbs every ROT-th op, plus the very last (slot, h=1)
                # so the pipeline drain runs on both engines in parallel
                if (2 * p + h) % ROT == ROT - 1 or \
                        (p == npair - 1 and h == 1):
                    pending_act.append((p, h))
                    return
                c2 = int(C2[p])
                ps2 = ps2_tiles.pop((p, h))
                hsc = hscpool.tile([128, 1024], bf16, tag=f"hsc{h}")
                # out = max(ps2, -b2c); accum_out = sum(out)
                #     = segment sum of max(ps2+b2c, 0) minus cap*b2c
                # (host adds the cap*b2c back)
                nc.vector.tensor_scalar(
                    two_region(hsc, c2), two_region(ps2, c2),
                    mbt[:, c:c + 1], 0.0,
                    op0=Alu.max, op1=Alu.add,
                    accum_out=outt[:, c:c + 1])

            def flush_act(upto):
                while pending_act and pending_act[0][0] <= upto:
                    p, h = pending_act.pop(0)
                    c2 = int(C2[p])
                    ps2 = ps2_tiles.pop((p, h))
                    c = h * npair + p
                    hsc = hscpool.tile([128, 1024], bf16, tag=f"hsc{h}")
                    nc.scalar.activation(
                        two_region(hsc, c2), two_region(ps2, c2), Act.Relu,
                        bias=b2ct[:, c:c + 1], accum_out=outt[:, c:c + 1])

            emit_l1(0)
            emit_relu1(0)
            emit_l1(1)
            emit_relu1(1)
            for p in range(1, npair):
                i0, i1 = 2 * p, 2 * p + 1
                if i0 % SB == 0:
                    load_super(i0 // SB + 1)
                flush_act(p - 2)
                emit_l1(i0)
                emit_relu1(i0)
                emit_l2(p - 1, 0)
                emit_relu2(p - 1, 0)
                emit_l1(i1)
                emit_relu1(i1)
                emit_l2(p - 1, 1)
                emit_relu2(p - 1, 1)
            emit_l2(npair - 1, 0)
            emit_relu2(npair - 1, 0)
            emit_l2(npair - 1, 1)
            emit_relu2(npair - 1, 1)
            flush_act(npair)

            nc.sync.dma_start(out_d[:], outt[:])

    nc.compile()
    return nc


def _get_program(nblk):
    if nblk not in _compiled:
        _compiled[nblk] = _build_program(nblk)
    return _compiled[nblk]


def _q8(x):
    return np.asarray(x, dtype=np.float32).astype(FP8)


def kernel(X, Z, W1, b1, W2, b2, W3, b3, cell_to_batch, sample_idx_batch):
    from scipy.special import erf

    X = np.asarray(X)
    Z = np.asarray(Z, dtype=np.float32)
    W1 = np.asarray(W1, dtype=np.float32)
    b1 = np.asarray(b1, dtype=np.float32)
    W2 = np.asarray(W2, dtype=np.float32)
    b2 = np.asarray(b2, dtype=np.float32)
    W3 = np.asarray(W3, dtype=np.float32)
    b3 = np.asarray(b3, dtype=np.float32)
    c2b = np.asarray(cell_to_batch).astype(np.int64)
    sib = np.asarray(sample_idx_batch).astype(np.int64)

    n = X.shape[0]
    nseg = sib.shape[0]
    seg = sib[c2b]
    npair = NSLOT
    assert nseg == N_CORES * NSLOT

    # ---- host layout prep -------------------------------------------------
    counts = np.bincount(seg, minlength=nseg).astype(np.int64)
    # per-core slot assignment: sort the core's segments by count descending
    seg2slot = np.zeros(nseg, dtype=np.int64)      # segment -> local slot
    slot2seg = np.zeros((N_CORES, NSLOT), dtype=np.int64)
    for c in range(N_CORES):
        segs = np.arange(c * NSLOT, (c + 1) * NSLOT)
        order_c = np.argsort(-counts[segs], kind="stable")
        slot2seg[c] = segs[order_c]
        seg2slot[segs[order_c]] = np.arange(NSLOT)
    seg_cap = CAPS[seg2slot]                       # [nseg] device capacity

    order = np.argsort(seg, kind="stable")
    seg_sorted = seg[order]
    run_starts = np.concatenate([[0], np.cumsum(counts)])[:nseg]
    ranks = np.arange(n, dtype=np.int64) - run_starts[seg_sorted]
    on_dev = ranks < seg_cap[seg_sorted]

    # column offset of each segment's data region inside its core's stream:
    # slot q data cols for sub-block b start at SUB_OFF[2q + b]
    xl8 = _q8(np.log1p(np.asarray(X, dtype=np.float32)))
    xt = np.zeros((N_CORES, DX, TOTCOL), dtype=FP8)
    one8 = np.float32(1.0).astype(FP8)
    c2_of_seg = (seg_cap // 2)
    # destination column for each on-device cell
    q_of_seg = seg2slot
    sub0 = SUB_OFF[2 * q_of_seg[seg_sorted]]       # per cell (via its seg)
    sub1 = SUB_OFF[2 * q_of_seg[seg_sorted] + 1]
    c2s = c2_of_seg[seg_sorted]
    in_b1 = ranks >= c2s
    col = np.where(in_b1, sub1 + (ranks - c2s), sub0 + ranks)
    core_of = seg_sorted // NSLOT
    od = on_dev
    xt[core_of[od], :, col[od]] = xl8[order[od]]
    # one-hot columns: for every slot/sub-block, cols [c2:2*c2] rows 2q,2q+1
    for q in range(NSLOT):
        c2q = int(C2[q])
        for b in range(2):
            o = int(SUB_OFF[2 * q + b])
            xt[:, 2 * q, o + c2q:o + 2 * c2q] = one8
            xt[:, 2 * q + 1, o + c2q:o + 2 * c2q] = one8

    # ---- weights ----------------------------------------------------------
    W1x = W1[:DX]
    w1_hi = _q8(W1x * WSCALE)
    w1dev = w1_hi.astype(np.float32) / WSCALE
    dW1 = w1dev - W1x
    w2_hi = _q8(W2 * WSCALE)
    dW2 = w2_hi.astype(np.float32) / WSCALE - W2

    w2hiq = np.zeros((128, 512), dtype=FP8)
    for h in range(2):
        for t in range(2):
            w2hiq[:, h * 256 + t * 128:h * 256 + (t + 1) * 128] = \
                w2_hi[t * 128:(t + 1) * 128, h * 128:(h + 1) * 128]

    # zb with the W1-quantization mean-correction, as fp8 hi+lo table rows
    xlf = xl8.astype(np.float32)
    mcol = xlf.mean(axis=0)
    vcol = xlf.var(axis=0)
    zb = Z @ W1[DX:DX + DZ] + b1 - mcol @ dW1    # [B, 256]
    zbs = zb * WSCALE
    zb_hi = _q8(zbs)
    zb_lo = _q8(zbs - zb_hi.astype(np.float32))
    zbq = (zb_hi.astype(np.float32) + zb_lo.astype(np.float32)) / WSCALE

    # per-core w1: [h][128, 256] = [64*W1x_hi | zb table]
    w1q = np.zeros((N_CORES, 2, 128, 256), dtype=FP8)
    for c in range(N_CORES):
        for h in range(2):
            w1q[c, h, :, 0:128] = w1_hi[:, h * 128:(h + 1) * 128]
            tab = np.zeros((128, 128), dtype=FP8)
            tab[2 * np.arange(NSLOT)] = zb_hi[slot2seg[c], h * 128:(h + 1) * 128]
            tab[2 * np.arange(NSLOT) + 1] = zb_lo[slot2seg[c], h * 128:(h + 1) * 128]
            w1q[c, h, :, 128:256] = tab

    # E[h1|seg] Gaussian closed form -> per-segment W2 mean-correction
    mu = mcol @ w1dev
    sig = np.sqrt(np.maximum(vcol @ (w1dev ** 2), 1e-12))
    muz = mu[None, :] + zbq
    u = muz / sig[None, :]
    Phi = 0.5 * (1.0 + erf(u / np.sqrt(2.0)))
    phi = np.exp(-0.5 * u * u) / np.sqrt(2.0 * np.pi)
    Eh1 = sig[None, :] * phi + muz * Phi
    b2c_seg = WSCALE * (b2[None, :] - Eh1 @ dW2)   # [B, 256]

    def per_slot_cols(src):                      # [B, 256] -> [core, 128, 2np]
        pv = src[slot2seg.reshape(-1)]           # core-major slot order
        a = pv.reshape(N_CORES, npair, 2, 128).transpose(0, 3, 2, 1)
        return np.ascontiguousarray(a).reshape(N_CORES, 128, 2 * npair)

    b2cd = per_slot_cols(b2c_seg)
    mbd = per_slot_cols(-b2c_seg)

    # ---- run on 8 cores ---------------------------------------------------
    nc = _get_program(NBLK)
    in_maps = []
    for c in range(N_CORES):
        in_maps.append({
            "xt": xt[c], "w1": w1q[c], "w2hi": w2hiq,
            "mb": mbd[c], "b2c": b2cd[c],
        })
    global _last_in_maps
    _last_in_maps = in_maps
    res = run_bass_kernel_spmd(nc, in_maps, list(range(N_CORES)))

    # ---- host epilogue ----------------------------------------------------
    per_core = []
    for c in range(N_CORES):
        o = res.results[c]["out"]               # [128, 2*npair]
        per_core.append(np.stack([o[:, 0:npair], o[:, npair:2 * npair]],
                                 axis=0))
    sums_slot = np.concatenate(per_core, axis=2)   # [2, 128, ncore*nslot]
    sums_slot = sums_slot.transpose(2, 0, 1).reshape(nseg, H)  # slot-major

    # map slot-major rows back to segment ids
    seg_of_row = slot2seg.reshape(-1)              # row r -> segment
    sums = np.zeros((nseg, H), dtype=np.float32)
    sums[seg_of_row] = sums_slot

    # DVE-computed (slot, half) columns used the shift identity: add the
    # cap*b2c they are missing.  ACT-computed ones are already complete.
    lq = seg2slot                                  # local slot of each segment
    dve_mask = np.stack(
        [((2 * lq + h) % ROT != ROT - 1) & ~((lq == npair - 1) & (h == 1))
         for h in range(2)], axis=1)               # [nseg, 2]
    shift = np.repeat(dve_mask, 128, axis=1) * \
        (seg_cap[:, None].astype(np.float32) * b2c_seg)
    sums = sums + shift

    # pad-cell contribution: xl = 0 -> h1 = fp8(relu(zbq))
    h1p = _q8(np.maximum(zbq, 0.0)).astype(np.float32)         # [B, 256]
    pre = h1p @ w2_hi.astype(np.float32) + b2c_seg
    h2p = np.maximum(pre, 0.0)                                 # [B, 256]

    real = np.minimum(counts, seg_cap)
    corr = (seg_cap - real).astype(np.float32)[:, None] * h2p

    S = (sums - corr) / WSCALE

    # host-evaluated overflow cells (exact f32 math)
    idx_host = order[~on_dev]
    if idx_host.shape[0]:
        hh1 = np.maximum(np.log1p(X[idx_host].astype(np.float32)) @ W1x
                         + (Z @ W1[DX:DX + DZ] + b1)[seg[idx_host]], 0.0)
        hh2 = np.maximum(hh1 @ W2 + b2, 0.0)
        np.add.at(S, seg[idx_host], hh2)

    denom = np.maximum(counts, 1).astype(np.float32)[:, None]
    Y = S @ W3 / denom + b3[None, :]
    Y[counts == 0] = 0.0
    return Y.astype(np.float32)
